# revision 22
# baseline (speedup 1.0000x reference)
"""Trainium2 Bass kernel for nn_DisentangledHead (disentangled attention head).

Reference computation (per batch element b):
    q_c = content[b] @ w_qc ; k_c = content[b] @ w_kc ; v = content[b] @ w_v
    q_p = position @ w_qp   ; k_p = position @ w_kp
    S   = (q_c k_c^T + q_p k_p^T) * scale          [T, T]
    attn = softmax(S, -1)                           [T, T]
    out  = attn @ v                                 [T, H]
Returns (out [B,T,H], attn [B,T,T]).

Sharding: data-parallel over B across the 8 NeuronCores (1 batch element
per core; position + weights replicated).

Kernel design per core (v2 - interleaved):
  - xpT [128, T] holds [content^T ; position^T] stacked on partitions,
    built with paired PE transposes (content tile i | position tile i).
  - Block-diagonal weights [[w_qc,0],[0,w_qp]] project xpT into
    qT/kT [128, T] = [q_c^T ; q_p^T] so a single K=128 matmul computes
    q_c k_c^T + q_p k_p^T.
  - Matmuls run in float32r (tf32) - 4x the fp32 rate; operand tiles are
    float32r so producers round once.
  - Main loop interleaves, per round r: pass A (S tile [128q, T] -> ACT
    exp(scale*S) with accum_out row sums -> DVE recip + normalize -> DMA
    attn rows) and pass B (S^T half-tiles -> ACT exp -> PE accumulates
    out^T = v^T exp(S^T), v stationary, col-tiled into a [128, 1024]
    PSUM accumulator). ACT is the bottleneck engine; everything else
    overlaps under it. PSUM: 4 (S) + 2 (St) + 2 (out^T) = 8 banks.
  - Epilogue: out^T -> PE transposes -> DVE scale by 1/rowsum -> DMA out.
"""

import numpy as np
from contextlib import ExitStack

import concourse.bass as bass
import concourse.tile as tile
from concourse import bacc, mybir
from concourse.bass_utils import run_bass_kernel_spmd
from concourse.masks import make_identity

F32 = mybir.dt.float32
F32R = mybir.dt.float32r  # tf32 matmul mode: 4x faster PE, ~2^-11 input rounding
AF = mybir.ActivationFunctionType

B = 8
T = 2048
C = 64
H = 64
P = 128
NT = T // P  # 16
NCORES = 8
SCALE = 1.0 / 8.0  # H ** -0.5

_INPUT_SPECS = [
    ("content", [T, C]),
    ("position", [T, C]),
    ("w_qc", [C, H]),
    ("w_kc", [C, H]),
    ("w_v", [C, H]),
    ("w_qp", [C, H]),
    ("w_kp", [C, H]),
]


def _emit(tc, ins, out_d, attn_d):
    nc = tc.nc
    with ExitStack() as ctx:
        consts = ctx.enter_context(tc.tile_pool(name="consts", bufs=1))
        persist = ctx.enter_context(tc.tile_pool(name="persist", bufs=1))

        ident = consts.tile([P, P], F32)
        make_identity(nc, ident)

        # fp32 weight staging + block-diagonal tf32 projection weights
        w_sb = {}
        for wname in ("w_qc", "w_kc", "w_v", "w_qp", "w_kp"):
            w_sb[wname] = consts.tile([C, H], F32, name=f"w_{wname}")
            nc.gpsimd.dma_start(out=w_sb[wname], in_=ins[wname])
        wq_stage = consts.tile([P, P], F32)
        wk_stage = consts.tile([P, P], F32)
        nc.vector.memset(wq_stage, 0.0)
        nc.vector.memset(wk_stage, 0.0)
        nc.vector.tensor_copy(wq_stage[0:C, 0:H], w_sb["w_qc"])
        nc.vector.tensor_copy(wq_stage[C:P, H:P], w_sb["w_qp"])
        nc.vector.tensor_copy(wk_stage[0:C, 0:H], w_sb["w_kc"])
        nc.vector.tensor_copy(wk_stage[C:P, H:P], w_sb["w_kp"])
        wq_blk = consts.tile([P, P], F32R)
        wk_blk = consts.tile([P, P], F32R)
        nc.vector.tensor_copy(wq_blk, wq_stage)
        nc.vector.tensor_copy(wk_blk, wk_stage)
        w_v_r = consts.tile([C, H], F32R)
        nc.vector.tensor_copy(w_v_r, w_sb["w_v"])

        # ---- xpT = [content^T ; position^T]  [128, T] --------------------
        # Stage both inputs with one DMA each: stage[p, 0, i, c] = content
        # row 128*i+p, stage[p, 1, i, c] = position row 128*i+p.
        xpT = persist.tile([P, T], F32R)
        with tc.tile_pool(name="tr_in", bufs=1) as tr_in, \
             tc.tile_pool(name="tr_ps", bufs=4, space="PSUM") as tr_ps:
            stage = tr_in.tile([P, NT, 2, C], F32)
            HN = NT // 2
            for g in range(2):
                nc.sync.dma_start(
                    out=stage[:, HN * g:HN * (g + 1), 0, :],
                    in_=ins["content"][HN * g * P:HN * (g + 1) * P, :].rearrange(
                        "(i p) c -> p i c", p=P))
                nc.sync.dma_start(
                    out=stage[:, HN * g:HN * (g + 1), 1, :],
                    in_=ins["position"][HN * g * P:HN * (g + 1) * P, :].rearrange(
                        "(i p) c -> p i c", p=P))
            for i in range(NT):
                pst = tr_ps.tile([P, P], F32)
                nc.tensor.transpose(pst, stage[:, i, :, :], ident)
                nc.vector.tensor_copy(xpT[:, P * i:P * (i + 1)], pst)

        # ---- projections -------------------------------------------------
        # qT rows 0:64 = (content @ w_qc)^T, rows 64:128 = (position @ w_qp)^T
        qT = persist.tile([P, T], F32R)
        kT = persist.tile([P, T], F32R)
        v_sb = persist.tile([P, NT, H], F32R)  # v row t=128*i+p at [p, i, :]

        with tc.tile_pool(name="pj_ps", bufs=2, space="PSUM") as pj_ps:
            for n in range(4):
                sl = slice(512 * n, 512 * (n + 1))
                psk = pj_ps.tile([P, 512], F32, tag="psk")
                nc.tensor.matmul(psk, lhsT=wk_blk, rhs=xpT[:, sl],
                                 start=True, stop=True)
                nc.scalar.copy(kT[:, sl], psk)
                psq = pj_ps.tile([P, 512], F32, tag="psq")
                nc.tensor.matmul(psq, lhsT=wq_blk, rhs=xpT[:, sl],
                                 start=True, stop=True)
                nc.vector.tensor_copy(qT[:, sl], psq)
            for i in range(NT):
                psv = pj_ps.tile([P, H], F32, tag="psv")
                nc.tensor.matmul(psv, lhsT=xpT[0:C, P * i:P * (i + 1)],
                                 rhs=w_v_r, start=True, stop=True)
                nc.vector.tensor_copy(v_sb[:, i, :], psv)

        # ---- interleaved main loop --------------------------------------
        sums2 = persist.tile([P, NT, 2], F32)
        sums = persist.tile([P, NT], F32)
        rsum = persist.tile([P, NT], F32)
        HT = T // 2

        with tc.tile_pool(name="ps_work", bufs=2, space="PSUM") as ps_work, \
             tc.tile_pool(name="ps_ot", bufs=1, space="PSUM") as ps_ot_pool, \
             tc.tile_pool(name="attn_sb", bufs=3) as attn_pool, \
             tc.tile_pool(name="est_sb", bufs=2) as est_pool:
            ps_ot = ps_ot_pool.tile([H, T], F32)
            LEAD = 2
            for r in range(NT + LEAD):
                if r < NT:
                    # pass B: S^T col-block r, two half-tiles -> out^T accum
                    rq = slice(P * r, P * (r + 1))
                    for hh in range(2):
                        ps2 = ps_work.tile([P, HT], F32, tag="work")
                        for n in range(2):
                            qs = slice(HT * hh + 512 * n, HT * hh + 512 * (n + 1))
                            nc.tensor.matmul(
                                ps2[:, 512 * n:512 * (n + 1)],
                                lhsT=kT[:, rq],
                                rhs=qT[:, qs], start=True, stop=True)
                        est = est_pool.tile([P, HT], F32R)
                        nc.scalar.activation(est, ps2, AF.Exp, scale=SCALE)
                        for n in range(2):
                            nc.tensor.matmul(
                                ps_ot[:, HT * hh + 512 * n:HT * hh + 512 * (n + 1)],
                                lhsT=v_sb[:, r, :],
                                rhs=est[:, 512 * n:512 * (n + 1)],
                                start=(r == 0), stop=(r == NT - 1))
                if r >= LEAD:
                    # pass A: S row-block rr -> attn rows
                    rr = r - LEAD
                    rq = slice(P * rr, P * (rr + 1))
                    et = attn_pool.tile([P, T], F32)
                    for hh in range(2):
                        ps = ps_work.tile([P, HT], F32, tag="work")
                        for n in range(2):
                            nc.tensor.matmul(
                                ps[:, 512 * n:512 * (n + 1)],
                                lhsT=qT[:, rq],
                                rhs=kT[:, HT * hh + 512 * n:HT * hh + 512 * (n + 1)],
                                start=True, stop=True)
                        nc.scalar.activation(et[:, HT * hh:HT * (hh + 1)], ps,
                                             AF.Exp, scale=SCALE,
                                             accum_out=sums2[:, rr, hh:hh + 1])
                    nc.vector.tensor_add(sums[:, rr:rr + 1], sums2[:, rr, 0:1],
                                         sums2[:, rr, 1:2])
                    nc.vector.reciprocal(rsum[:, rr:rr + 1], sums[:, rr:rr + 1])
                    nc.vector.tensor_scalar_mul(et, et, rsum[:, rr:rr + 1])
                    nc.sync.dma_start(out=attn_d[P * rr:P * (rr + 1), :], in_=et)

            # Epilogue: out^T[h, q] * rsum[q%128, q//128].
            # Flatten rsum partitions-major into one row (rrow[0, 16p+i] =
            # rsum[p, i]), gpsimd-broadcast it across the 64 h-partitions,
            # then one PSUM-read multiply with the free axis permuted
            # (q = 128i + p reads rb[h, 16p + i]) produces out^T in SBUF.
            with tc.tile_pool(name="ep_sb", bufs=1) as ep_sb:
                rrow = ep_sb.tile([1, T], F32)
                nc.sync.dma_start(out=rrow, in_=rsum)
                rb = ep_sb.tile([H, T], F32)
                nc.gpsimd.partition_broadcast(rb, rrow)
                outT_sb = ep_sb.tile([H, T], F32)
                nc.vector.tensor_mul(
                    outT_sb.rearrange("h (i p) -> h i p", i=NT),
                    ps_ot.rearrange("h (i p) -> h i p", i=NT),
                    rb.rearrange("h (p i) -> h i p", i=NT))
                nc.sync.dma_start(out=out_d, in_=outT_sb)


def build_program():
    nc = bacc.Bacc("TRN2", target_bir_lowering=False, debug=False,
                   num_devices=NCORES)
    ins = {}
    for name, shape in _INPUT_SPECS:
        ins[name] = nc.dram_tensor(name, shape, F32, kind="ExternalInput").ap()
    attn_d = nc.dram_tensor("attn", [T, T], F32, kind="ExternalOutput").ap()
    out_d = nc.dram_tensor("out", [H, T], F32, kind="ExternalOutput").ap()
    with tile.TileContext(nc) as tc:
        _emit(tc, ins, out_d, attn_d)
    nc.compile()
    return nc


_PROGRAM = None


def _get_program():
    global _PROGRAM
    if _PROGRAM is None:
        _PROGRAM = build_program()
    return _PROGRAM


def make_in_maps(content, position, w_qc, w_kc, w_v, w_qp, w_kp):
    common = {
        "position": np.ascontiguousarray(position, dtype=np.float32),
        "w_qc": np.ascontiguousarray(w_qc, dtype=np.float32),
        "w_kc": np.ascontiguousarray(w_kc, dtype=np.float32),
        "w_v": np.ascontiguousarray(w_v, dtype=np.float32),
        "w_qp": np.ascontiguousarray(w_qp, dtype=np.float32),
        "w_kp": np.ascontiguousarray(w_kp, dtype=np.float32),
    }
    return [
        {"content": np.ascontiguousarray(content[b], dtype=np.float32), **common}
        for b in range(B)
    ]


def run(inputs, trace=False):
    nc = _get_program()
    in_maps = make_in_maps(**{k: np.asarray(v) for k, v in inputs.items()})
    res = run_bass_kernel_spmd(nc, in_maps, list(range(NCORES)), trace=trace)
    out = np.stack([np.asarray(res.results[b]["out"]).T for b in range(B)])
    attn = np.stack([np.asarray(res.results[b]["attn"]) for b in range(B)])
    return (out, attn), res


def kernel(**inputs):
    (out, attn), _ = run(inputs, trace=False)
    return out, attn


# revision 24
# speedup vs baseline: 1.0573x; 1.0573x over previous
"""Trainium2 Bass kernel for nn_DisentangledHead (disentangled attention head).

Reference computation (per batch element b):
    q_c = content[b] @ w_qc ; k_c = content[b] @ w_kc ; v = content[b] @ w_v
    q_p = position @ w_qp   ; k_p = position @ w_kp
    S   = (q_c k_c^T + q_p k_p^T) * scale          [T, T]
    attn = softmax(S, -1)                           [T, T]
    out  = attn @ v                                 [T, H]
Returns (out [B,T,H], attn [B,T,T]).

Sharding: data-parallel over B across the 8 NeuronCores (1 batch element
per core; position + weights replicated).

Kernel design per core (v2 - interleaved):
  - xpT [128, T] holds [content^T ; position^T] stacked on partitions,
    built with paired PE transposes (content tile i | position tile i).
  - Block-diagonal weights [[w_qc,0],[0,w_qp]] project xpT into
    qT/kT [128, T] = [q_c^T ; q_p^T] so a single K=128 matmul computes
    q_c k_c^T + q_p k_p^T.
  - Matmuls run in float32r (tf32) - 4x the fp32 rate; operand tiles are
    float32r so producers round once.
  - Main loop interleaves, per round r: pass A (S tile [128q, T] -> ACT
    exp(scale*S) with accum_out row sums -> DVE recip + normalize -> DMA
    attn rows) and pass B (S^T half-tiles -> ACT exp -> PE accumulates
    out^T = v^T exp(S^T), v stationary, col-tiled into a [128, 1024]
    PSUM accumulator). ACT is the bottleneck engine; everything else
    overlaps under it. PSUM: 4 (S) + 2 (St) + 2 (out^T) = 8 banks.
  - Epilogue: out^T -> PE transposes -> DVE scale by 1/rowsum -> DMA out.
"""

import numpy as np
from contextlib import ExitStack

import concourse.bass as bass
import concourse.tile as tile
from concourse import bacc, mybir
from concourse.bass_utils import run_bass_kernel_spmd
from concourse.masks import make_identity

F32 = mybir.dt.float32
F32R = mybir.dt.float32r  # tf32 matmul mode: 4x faster PE, ~2^-11 input rounding
AF = mybir.ActivationFunctionType

B = 8
T = 2048
C = 64
H = 64
P = 128
NT = T // P  # 16
NCORES = 8
SCALE = 1.0 / 8.0  # H ** -0.5

_INPUT_SPECS = [
    ("content", [T, C]),
    ("position", [T, C]),
    ("w_qc", [C, H]),
    ("w_kc", [C, H]),
    ("w_v", [C, H]),
    ("w_qp", [C, H]),
    ("w_kp", [C, H]),
]


def _emit(tc, ins, out_d, attn_d):
    nc = tc.nc
    with ExitStack() as ctx:
        consts = ctx.enter_context(tc.tile_pool(name="consts", bufs=1))
        persist = ctx.enter_context(tc.tile_pool(name="persist", bufs=1))
        stage_pool = ctx.enter_context(tc.tile_pool(name="tr_in", bufs=1))

        # kick the input staging DMAs before anything else
        stage = stage_pool.tile([P, NT, 2, C], F32)
        HN = NT // 2
        for g in range(2):
            nc.sync.dma_start(
                out=stage[:, HN * g:HN * (g + 1), 0, :],
                in_=ins["content"][HN * g * P:HN * (g + 1) * P, :].rearrange(
                    "(i p) c -> p i c", p=P))
            nc.sync.dma_start(
                out=stage[:, HN * g:HN * (g + 1), 1, :],
                in_=ins["position"][HN * g * P:HN * (g + 1) * P, :].rearrange(
                    "(i p) c -> p i c", p=P))

        ident = consts.tile([P, P], F32)
        make_identity(nc, ident)

        # fp32 weight staging + block-diagonal tf32 projection weights
        w_sb = {}
        for wname in ("w_qc", "w_kc", "w_v", "w_qp", "w_kp"):
            w_sb[wname] = consts.tile([C, H], F32, name=f"w_{wname}")
            nc.gpsimd.dma_start(out=w_sb[wname], in_=ins[wname])
        wq_stage = consts.tile([P, P], F32)
        wk_stage = consts.tile([P, P], F32)
        nc.vector.memset(wq_stage, 0.0)
        nc.vector.memset(wk_stage, 0.0)
        nc.vector.tensor_copy(wq_stage[0:C, 0:H], w_sb["w_qc"])
        nc.vector.tensor_copy(wq_stage[C:P, H:P], w_sb["w_qp"])
        nc.vector.tensor_copy(wk_stage[0:C, 0:H], w_sb["w_kc"])
        nc.vector.tensor_copy(wk_stage[C:P, H:P], w_sb["w_kp"])
        wq_blk = consts.tile([P, P], F32R)
        wk_blk = consts.tile([P, P], F32R)
        nc.vector.tensor_copy(wq_blk, wq_stage)
        nc.vector.tensor_copy(wk_blk, wk_stage)
        w_v_r = consts.tile([C, H], F32R)
        nc.vector.tensor_copy(w_v_r, w_sb["w_v"])

        # ---- xpT = [content^T ; position^T]  [128, T] --------------------
        xpT = persist.tile([P, T], F32R)
        with tc.tile_pool(name="tr_ps", bufs=4, space="PSUM") as tr_ps:
            # dummy matmuls while the staging DMAs land: pulls the PE HAM
            # clock-gate to 2.4 GHz before the real work
            warm = tr_ps.tile([P, P], F32, tag="warm")
            for _ in range(14):
                nc.tensor.matmul(warm, lhsT=ident, rhs=ident,
                                 start=True, stop=True)
            for i in range(NT):
                pst = tr_ps.tile([P, P], F32)
                nc.tensor.transpose(pst, stage[:, i, :, :], ident)
                nc.vector.tensor_copy(xpT[:, P * i:P * (i + 1)], pst)

        # ---- projections -------------------------------------------------
        # qT rows 0:64 = (content @ w_qc)^T, rows 64:128 = (position @ w_qp)^T
        qT = persist.tile([P, T], F32R)
        kT = persist.tile([P, T], F32R)
        v_sb = persist.tile([P, NT, H], F32R)  # v row t=128*i+p at [p, i, :]

        with tc.tile_pool(name="pj_ps", bufs=2, space="PSUM") as pj_ps:
            for n in range(4):
                sl = slice(512 * n, 512 * (n + 1))
                psk = pj_ps.tile([P, 512], F32, tag="psk")
                nc.tensor.matmul(psk, lhsT=wk_blk, rhs=xpT[:, sl],
                                 start=True, stop=True)
                nc.scalar.copy(kT[:, sl], psk)
                psq = pj_ps.tile([P, 512], F32, tag="psq")
                nc.tensor.matmul(psq, lhsT=wq_blk, rhs=xpT[:, sl],
                                 start=True, stop=True)
                nc.vector.tensor_copy(qT[:, sl], psq)
            for i in range(NT):
                psv = pj_ps.tile([P, H], F32, tag="psv")
                nc.tensor.matmul(psv, lhsT=xpT[0:C, P * i:P * (i + 1)],
                                 rhs=w_v_r, start=True, stop=True)
                nc.vector.tensor_copy(v_sb[:, i, :], psv)

        # ---- interleaved main loop --------------------------------------
        sums2 = persist.tile([P, NT, 2], F32)
        sums = persist.tile([P, NT], F32)
        rsum = persist.tile([P, NT], F32)
        HT = T // 2

        HNT = NT // 2
        with tc.tile_pool(name="ps_work", bufs=2, space="PSUM") as ps_work, \
             tc.tile_pool(name="ps_ot", bufs=1, space="PSUM") as ps_ot_pool, \
             tc.tile_pool(name="attn_sb", bufs=3) as attn_pool, \
             tc.tile_pool(name="est_sb", bufs=2) as est_pool, \
             tc.tile_pool(name="ep_sb", bufs=2) as ep_sb:
            ps_ot = ps_ot_pool.tile([H, T], F32)
            LEAD = 2
            rb_half = [None, None]
            for r in range(NT + LEAD):
                if r < NT:
                    # pass B: S^T col-block r, two half-tiles -> out^T accum
                    rq = slice(P * r, P * (r + 1))
                    for hh in range(2):
                        ps2 = ps_work.tile([P, HT], F32, tag="work")
                        for n in range(2):
                            qs = slice(HT * hh + 512 * n, HT * hh + 512 * (n + 1))
                            nc.tensor.matmul(
                                ps2[:, 512 * n:512 * (n + 1)],
                                lhsT=kT[:, rq],
                                rhs=qT[:, qs], start=True, stop=True)
                        est = est_pool.tile([P, HT], F32R)
                        nc.scalar.activation(est, ps2, AF.Exp, scale=SCALE)
                        for n in range(2):
                            nc.tensor.matmul(
                                ps_ot[:, HT * hh + 512 * n:HT * hh + 512 * (n + 1)],
                                lhsT=v_sb[:, r, :],
                                rhs=est[:, 512 * n:512 * (n + 1)],
                                start=(r == 0), stop=(r == NT - 1))
                if r >= LEAD:
                    # pass A: S row-block rr -> attn rows
                    rr = r - LEAD
                    rq = slice(P * rr, P * (rr + 1))
                    et = attn_pool.tile([P, T], F32)
                    for hh in range(2):
                        ps = ps_work.tile([P, HT], F32, tag="work")
                        for n in range(2):
                            nc.tensor.matmul(
                                ps[:, 512 * n:512 * (n + 1)],
                                lhsT=qT[:, rq],
                                rhs=kT[:, HT * hh + 512 * n:HT * hh + 512 * (n + 1)],
                                start=True, stop=True)
                        nc.scalar.activation(et[:, HT * hh:HT * (hh + 1)], ps,
                                             AF.Exp, scale=SCALE,
                                             accum_out=sums2[:, rr, hh:hh + 1])
                    nc.vector.tensor_add(sums[:, rr:rr + 1], sums2[:, rr, 0:1],
                                         sums2[:, rr, 1:2])
                    nc.vector.reciprocal(rsum[:, rr:rr + 1], sums[:, rr:rr + 1])
                    nc.vector.tensor_scalar_mul(et, et, rsum[:, rr:rr + 1])
                    nc.sync.dma_start(out=attn_d[P * rr:P * (rr + 1), :], in_=et)
                    # out^T normalization, by q-halves: rrow[0, 8p+i] =
                    # rsum[p, g*8+i]; broadcast across h partitions; multiply
                    # out^T (free axis permuted: q = 128i + p reads 8p+i).
                    # prep the reciprocal row + broadcast as soon as the
                    # needed rsum half exists; the multiplies wait for the
                    # ps_ot accumulation to stop (r == NT-1 of pass B).
                    if rr in (HNT - 1, NT - 1):
                        g = 0 if rr == HNT - 1 else 1
                        rrow = ep_sb.tile([1, T // 2], F32, tag="rrow")
                        nc.sync.dma_start(out=rrow,
                                          in_=rsum[:, HNT * g:HNT * (g + 1)])
                        rb = ep_sb.tile([H, T // 2], F32, tag="rb")
                        nc.gpsimd.partition_broadcast(rb, rrow)
                        rb_half[g] = rb
                    if rr in (NT - 2, NT - 1):
                        g = rr - (NT - 2)
                        qh = slice(T // 2 * g, T // 2 * (g + 1))
                        outT_sb = ep_sb.tile([H, T // 2], F32, tag="outT")
                        nc.vector.tensor_mul(
                            outT_sb.rearrange("h (i p) -> h i p", i=HNT),
                            ps_ot[:, qh].rearrange("h (i p) -> h i p", i=HNT),
                            rb_half[g].rearrange("h (p i) -> h i p", i=HNT))
                        nc.sync.dma_start(out=out_d[:, qh], in_=outT_sb)

            # Epilogue handled inside the round loop (split in q-halves).


def build_program():
    nc = bacc.Bacc("TRN2", target_bir_lowering=False, debug=False,
                   num_devices=NCORES)
    ins = {}
    for name, shape in _INPUT_SPECS:
        ins[name] = nc.dram_tensor(name, shape, F32, kind="ExternalInput").ap()
    attn_d = nc.dram_tensor("attn", [T, T], F32, kind="ExternalOutput").ap()
    out_d = nc.dram_tensor("out", [H, T], F32, kind="ExternalOutput").ap()
    with tile.TileContext(nc) as tc:
        _emit(tc, ins, out_d, attn_d)
    nc.compile()
    return nc


_PROGRAM = None


def _get_program():
    global _PROGRAM
    if _PROGRAM is None:
        _PROGRAM = build_program()
    return _PROGRAM


def make_in_maps(content, position, w_qc, w_kc, w_v, w_qp, w_kp):
    common = {
        "position": np.ascontiguousarray(position, dtype=np.float32),
        "w_qc": np.ascontiguousarray(w_qc, dtype=np.float32),
        "w_kc": np.ascontiguousarray(w_kc, dtype=np.float32),
        "w_v": np.ascontiguousarray(w_v, dtype=np.float32),
        "w_qp": np.ascontiguousarray(w_qp, dtype=np.float32),
        "w_kp": np.ascontiguousarray(w_kp, dtype=np.float32),
    }
    return [
        {"content": np.ascontiguousarray(content[b], dtype=np.float32), **common}
        for b in range(B)
    ]


def run(inputs, trace=False):
    nc = _get_program()
    in_maps = make_in_maps(**{k: np.asarray(v) for k, v in inputs.items()})
    res = run_bass_kernel_spmd(nc, in_maps, list(range(NCORES)), trace=trace)
    out = np.stack([np.asarray(res.results[b]["out"]).T for b in range(B)])
    attn = np.stack([np.asarray(res.results[b]["attn"]) for b in range(B)])
    return (out, attn), res


def kernel(**inputs):
    (out, attn), _ = run(inputs, trace=False)
    return out, attn
